# revision 1
# baseline (speedup 1.0000x reference)
"""Trainium2 Bass kernel for the BaseMemory coref scoring module.

Computes, for full inputs (M=65536 memory slots, D=768, E=20, H=64):
    score = relu(pair @ W1 + b1) @ W2 + b2, masked with ent_counter>0,
    where pair = [mem, ment, mem*ment, dist_emb, cnt_emb].

Sharding: data-parallel over the cluster dimension M across 8 NeuronCores.
Each core's shard of mem_vectors is laid out [D, MS] (contraction-major) so
the PE consumes it directly; all FLOPs and all HBM traffic stay on device.

Key algebraic folds (host side, O(D*H) work):
  - mem@W1_mem + (mem*ment)@W1_had = mem @ (W1_mem + diag(ment)@W1_had)
  - ment@W1_ment + b1 folded into the 10-row dist bucket table
  - bucket embedding lookups become one-hot rows contracted on the PE
  - masking folded into the PE accumulation (exact)
"""

import os
import numpy as np

# The bass kernel executes through the axon PJRT backend; make sure jax can
# see it even if the caller pinned JAX_PLATFORMS (e.g. to "cpu").
_jp = os.environ.get("JAX_PLATFORMS")
if _jp is not None and _jp != "" and "axon" not in _jp:
    os.environ["JAX_PLATFORMS"] = "axon," + _jp

M, D, E, H = 65536, 768, 20, 64
N_CORES = 8
MS = M // N_CORES          # rows per core = 8192
GROUP = 512                # rows per PE matmul group
N_GROUPS = MS // GROUP     # 16
SG = 4                     # groups per DMA super-group
N_SG = N_GROUPS // SG      # 4
KCH = D // 128             # 6 contraction chunks
NF = 22                    # 10 dist onehot, 10 cnt onehot, notmask, ones
N_BLK = MS // 128          # 64 feature blocks per core
BIG = float(2 ** 20)       # pre-relu kill value for masked rows

_CACHE = {}


def _build():
    """Build + compile the 8-core SPMD bass program once per process."""
    if "nc" in _CACHE:
        return _CACHE["nc"]

    import concourse.bass as bass
    import concourse.mybir as mybir
    import concourse.tile as tile
    from concourse import bacc
    from concourse.masks import make_identity

    F32 = mybir.dt.float32
    F32R = mybir.dt.float32r

    nc = bacc.Bacc("TRN2", target_bir_lowering=False, debug=False,
                   enable_asserts=False, num_devices=N_CORES)

    xt_d = nc.dram_tensor("xt", [D, MS], F32R, kind="ExternalInput").ap()
    lms_d = nc.dram_tensor("lms", [128, N_BLK], F32, kind="ExternalInput").ap()
    cnt_d = nc.dram_tensor("cnt", [128, N_BLK], F32, kind="ExternalInput").ap()
    w1_d = nc.dram_tensor("w1", [D, H], F32R, kind="ExternalInput").ap()
    tcat_d = nc.dram_tensor("tcat", [NF, H], F32R, kind="ExternalInput").ap()
    wsc_d = nc.dram_tensor("wsc", [H + NF, 1], F32R, kind="ExternalInput").ap()
    lo_d = nc.dram_tensor("lo", [128, NF], F32, kind="ExternalInput").ap()
    hi_d = nc.dram_tensor("hi", [128, NF], F32, kind="ExternalInput").ap()
    out_d = nc.dram_tensor("out", [MS], F32, kind="ExternalOutput").ap()

    # xt[d, m]: tile (k, s) = [128, SG*GROUP] at rows 128k, cols 2048s
    xt_r = xt_d.rearrange("(kp k2 p) (s c) -> p kp k2 s c", p=128, k2=2,
                          s=N_SG)
    w1_r = w1_d.rearrange("(k p) n -> p k n", p=128)    # [128, 6, 64]
    out_r = out_d.rearrange("(s c) -> s c", s=N_SG)  # [4, 2048]

    ge = mybir.AluOpType.is_ge
    le = mybir.AluOpType.is_le
    relu = mybir.ActivationFunctionType.Relu

    with tile.TileContext(nc) as tc:
        with (
            tc.tile_pool(name="consts", bufs=1) as cpool,
            tc.tile_pool(name="feat", bufs=1) as fpool,
            tc.tile_pool(name="xin", bufs=8) as px,
            tc.tile_pool(name="ht", bufs=6) as pht,
            tc.tile_pool(name="osb", bufs=2) as posb,
            tc.tile_pool(name="psf", bufs=2, space="PSUM") as psf,
            tc.tile_pool(name="psz", bufs=4, space="PSUM") as psz,
            tc.tile_pool(name="pss", bufs=2, space="PSUM") as pss,
        ):
            # consts issue on the scalar HWDGE queue so the big xt DMAs
            # (sync queue) start immediately
            ident_t = cpool.tile([128, 128], F32, tag="ident")
            make_identity(nc, ident_t[:])
            ident_r = cpool.tile([128, 128], F32R, tag="identr")
            nc.vector.tensor_copy(ident_r[:], ident_t[:])
            ident = ident_r[:]

            w1t = cpool.tile([128, KCH, H], F32R, tag="w1t")
            nc.scalar.dma_start(w1t[:], w1_r[:])
            lo_t = cpool.tile([128, NF], F32, tag="lo")
            nc.scalar.dma_start(lo_t[:], lo_d[:])
            hi_t = cpool.tile([128, NF], F32, tag="hi")
            nc.scalar.dma_start(hi_t[:], hi_d[:])
            lms_t = cpool.tile([128, N_BLK], F32, tag="lms")
            nc.scalar.dma_start(lms_t[:], lms_d[:])
            cnt_t = cpool.tile([128, N_BLK], F32, tag="cnt")
            nc.scalar.dma_start(cnt_t[:], cnt_d[:])
            tcat_full = cpool.tile([H + NF, H], F32R, tag="tcat")
            tcat = tcat_full[H:H + NF, :]
            nc.scalar.dma_start(tcat, tcat_d[:])
            wsc = cpool.tile([H + NF, 1], F32R, tag="wsc")
            nc.scalar.dma_start(wsc[:], wsc_d[:])

            # F[p, b, i] = onehot / mask features for row m = 128b + p
            tge = fpool.tile([128, N_BLK, NF], F32, tag="tge")
            tle = fpool.tile([128, N_BLK, NF], F32, tag="tle")
            fall = fpool.tile([128, N_BLK, NF], F32R, tag="fall")
            lms_b = lms_t[:, :, None].broadcast_to([128, N_BLK, 10])
            cnt_b = cnt_t[:, :, None].broadcast_to([128, N_BLK, 12])
            nc.vector.tensor_tensor(
                tge[:, :, 0:10], lms_b,
                lo_t[:, None, 0:10].broadcast_to([128, N_BLK, 10]), ge)
            nc.vector.tensor_tensor(
                tge[:, :, 10:NF], cnt_b,
                lo_t[:, None, 10:NF].broadcast_to([128, N_BLK, 12]), ge)
            nc.vector.tensor_tensor(
                tle[:, :, 0:10], lms_b,
                hi_t[:, None, 0:10].broadcast_to([128, N_BLK, 10]), le)
            nc.vector.tensor_tensor(
                tle[:, :, 10:NF], cnt_b,
                hi_t[:, None, 10:NF].broadcast_to([128, N_BLK, 12]), le)
            nc.vector.tensor_mul(fall[:], tge[:], tle[:])

            osb_tiles = {}
            pending = None

            def emit_score(g, ht):
                sc = pss.tile([1, GROUP], F32, tag="pss")
                nc.tensor.matmul(sc[:], wsc[:], ht[:], start=True, stop=True)
                sq = g // SG
                if g % SG == 0:
                    osb_t = posb.tile([1, SG * GROUP], F32, tag="osb")
                    osb_tiles[sq] = osb_t
                orow = osb_tiles[sq][0:1, GROUP * (g % SG):GROUP * (g % SG + 1)]
                if g % 2 == 0:
                    nc.vector.tensor_copy(orow, sc[:])
                else:
                    nc.scalar.copy(orow, sc[:])
                if g % SG == SG - 1:
                    nc.gpsimd.dma_start(out_r[sq:sq + 1, :],
                                        osb_tiles.pop(sq)[:])

            def load_sg(s):
                xts = []
                for kp in range(KCH // 2):
                    xk = px.tile([128, 2, SG * GROUP], F32R, tag="xin")
                    if s == 0:
                        # split so group 0's chunks land first
                        nc.sync.dma_start(xk[:, :, 0:GROUP],
                                          xt_r[:, kp, :, s, 0:GROUP])
                        nc.sync.dma_start(xk[:, :, GROUP:],
                                          xt_r[:, kp, :, s, GROUP:])
                    else:
                        nc.sync.dma_start(xk[:], xt_r[:, kp, :, s, :])
                    xts.append(xk)
                return xts

            sg_tiles = {0: load_sg(0), 1: load_sg(1)}
            for s in range(N_SG):
                if s + 2 < N_SG:
                    sg_tiles[s + 2] = load_sg(s + 2)
                xts = sg_tiles.pop(s)
                for gi in range(SG):
                    g = SG * s + gi
                    off = GROUP * gi
                    if pending is not None:
                        emit_score(*pending)

                    zt = psz.tile([H, GROUP], F32, tag="psz")
                    for k in range(KCH):
                        nc.tensor.matmul(zt[:], w1t[:, k, :],
                                         xts[k // 2][:, k % 2,
                                                     off:off + GROUP],
                                         start=(k == 0), stop=False)

                    # transpose the 4 feature blocks of this group
                    psft = psf.tile([NF, GROUP], F32R, tag="psf")
                    for j in range(4):
                        b = 4 * g + j
                        nc.tensor.transpose(
                            psft[:, 128 * j:128 * (j + 1)],
                            fall[:, b, :], ident)
                    # ht rows 0..63 = relu(z.T), rows 64..85 = F.T
                    ht = pht.tile([H + NF, GROUP], F32R, tag="ht")
                    if g % 2 == 0:
                        nc.vector.tensor_copy(ht[H:H + NF, :], psft[:])
                    else:
                        nc.scalar.copy(ht[H:H + NF, :], psft[:])

                    nc.tensor.matmul(zt[:], tcat, ht[H:H + NF, :],
                                     start=False, stop=True)

                    nc.scalar.activation(ht[0:H, :], zt[:], relu)
                    pending = (g, ht)
                if s == N_SG - 1:
                    emit_score(*pending)
                    pending = None

    nc.compile()
    _CACHE["nc"] = nc
    return nc


def _prepare_maps(ment_emb, mem_vectors, dist_table, counter_table,
                  W1, b1, W2, b2, ent_counter, last_mention_start, ment_start):
    f32 = np.float32
    ment = np.asarray(ment_emb, f32)
    mem = np.asarray(mem_vectors, f32)
    W1 = np.asarray(W1, f32)
    ms = float(np.asarray(ment_start).astype(np.float64))

    W1m, W1r, W1h = W1[0:D], W1[D:2 * D], W1[2 * D:3 * D]
    W1d, W1c = W1[3 * D:3 * D + E], W1[3 * D + E:3 * D + 2 * E]

    w1eff = (W1m + ment[:, None] * W1h).astype(f32)              # [768, 64]
    bias_vec = (np.asarray(b1, f32) + ment @ W1r).astype(f32)    # [64]
    T_d = (np.asarray(dist_table, f32) @ W1d + bias_vec).astype(f32)
    T_c = (np.asarray(counter_table, f32) @ W1c).astype(f32)
    b2v = float(np.asarray(b2, f32).reshape(-1)[0])

    tcat = np.concatenate(
        [T_d, T_c, np.full((1, H), -BIG, f32), np.zeros((1, H), f32)], 0)
    # single score matmul: rows 0..63 act on relu(z.T), rows 64..85 on F.T
    wsc = np.zeros((H + NF, 1), f32)
    wsc[0:H, 0] = np.asarray(W2, f32).reshape(-1)
    wsc[H + 20, 0] = -10000.0 - b2v
    wsc[H + 21, 0] = b2v

    # bucket i covers c in [A[i], B[i]] (identity below 5, log2 above, clip 9)
    A = np.array([-1e9, 1, 2, 3, 4, 5, 8, 16, 32, 64], np.float64)
    B = np.array([0, 1, 2, 3, 4, 7, 15, 31, 63, 1e9], np.float64)
    # dist bucket in lms terms: dist = ms - lms in [A,B] <=> lms in [ms-B, ms-A]
    lo = np.concatenate([ms - B, A, [-1e9], [-1e9]]).astype(f32)
    hi = np.concatenate([ms - A, B, [0.0], [1e9]]).astype(f32)
    lo_rep = np.ascontiguousarray(np.broadcast_to(lo, (128, NF)))
    hi_rep = np.ascontiguousarray(np.broadcast_to(hi, (128, NF)))

    lms_f = np.asarray(last_mention_start).astype(f32)
    cnt_f = np.asarray(ent_counter).astype(f32)

    in_maps = []
    for c in range(N_CORES):
        sl = slice(c * MS, (c + 1) * MS)
        in_maps.append(dict(
            xt=np.ascontiguousarray(mem[sl].T),
            lms=np.ascontiguousarray(lms_f[sl].reshape(N_BLK, 128).T),
            cnt=np.ascontiguousarray(cnt_f[sl].reshape(N_BLK, 128).T),
            w1=w1eff, tcat=tcat, wsc=wsc, lo=lo_rep, hi=hi_rep))
    return in_maps


def _postprocess(results):
    out = np.empty(M + 1, np.float32)
    for c in range(N_CORES):
        out[c * MS:(c + 1) * MS] = results[c]["out"]
    out[M] = 0.0
    return out


def run_spmd(in_maps, trace=False):
    from concourse.bass_utils import run_bass_kernel_spmd
    nc = _build()
    return run_bass_kernel_spmd(nc, in_maps, list(range(N_CORES)), trace=trace)


def kernel(**inputs):
    in_maps = _prepare_maps(**inputs)
    res = run_spmd(in_maps, trace=False)
    return _postprocess(res.results)



# revision 2
# speedup vs baseline: 1.6275x; 1.6275x over previous
"""Trainium2 Bass kernel for the BaseMemory coref scoring module.

Computes, for full inputs (M=65536 memory slots, D=768, E=20, H=64):
    score = relu(pair @ W1 + b1) @ W2 + b2, masked with ent_counter>0,
    where pair = [mem, ment, mem*ment, dist_emb, cnt_emb].

Sharding: data-parallel over the cluster dimension M across 8 NeuronCores.
Each core's shard of mem_vectors is laid out [D, MS] (contraction-major) so
the PE consumes it directly; all FLOPs and all HBM traffic stay on device.

Key folds (host side, O(D*H) + O(M) work on the small tensors only):
  - mem@W1_mem + (mem*ment)@W1_had = mem @ (W1_mem + diag(ment)@W1_had)
  - ment@W1_ment + b1 folded into the 10-row dist bucket table
  - bucket one-hots precomputed on host (O(M) int compares) and streamed
    as a [22, MS] bf16 plane; contracted on the PE against the folded
    10-row tables (masking folded into the PE accumulation, exact)
  - mem_vectors streamed as bf16: halves HBM traffic (the roofline term);
    all accumulation stays fp32 in PSUM
"""

import os
import numpy as np

# The bass kernel executes through the axon PJRT backend; make sure jax can
# see it even if the caller pinned JAX_PLATFORMS (e.g. to "cpu").
_jp = os.environ.get("JAX_PLATFORMS")
if _jp is not None and _jp != "" and "axon" not in _jp:
    os.environ["JAX_PLATFORMS"] = "axon," + _jp

M, D, E, H = 65536, 768, 20, 64
N_CORES = 8
MS = M // N_CORES          # rows per core = 8192
GROUP = 512                # rows per PE matmul group
N_GROUPS = MS // GROUP     # 16
SG = 4                     # groups per DMA super-group
N_SG = N_GROUPS // SG      # 4
KCH = D // 128             # 6 contraction chunks
NF = 22                    # 10 dist onehot, 10 cnt onehot, notmask, ones
HT = H + NF                # 86 rows of the score-matmul rhs
BIG = float(2 ** 20)       # pre-relu kill value for masked rows

_CACHE = {}


def _build():
    """Build + compile the 8-core SPMD bass program once per process."""
    if "nc" in _CACHE:
        return _CACHE["nc"]

    import concourse.bass as bass
    import concourse.mybir as mybir
    import concourse.tile as tile
    from concourse import bacc

    F32 = mybir.dt.float32
    BF16 = mybir.dt.bfloat16

    nc = bacc.Bacc("TRN2", target_bir_lowering=False, debug=False,
                   enable_asserts=False, num_devices=N_CORES)

    xt_d = nc.dram_tensor("xt", [D, MS], BF16, kind="ExternalInput").ap()
    oh_d = nc.dram_tensor("oh", [NF, MS], BF16, kind="ExternalInput").ap()
    w1_d = nc.dram_tensor("w1", [D, H], BF16, kind="ExternalInput").ap()
    tcat_d = nc.dram_tensor("tcat", [NF, H], BF16, kind="ExternalInput").ap()
    wsc_d = nc.dram_tensor("wsc", [HT, 1], BF16, kind="ExternalInput").ap()
    out_d = nc.dram_tensor("out", [MS], F32, kind="ExternalOutput").ap()

    # xt[d, m]: tile (k, s) = [128, SG*GROUP] at rows 128k, cols 2048s
    xt_r = xt_d.rearrange("(kp k2 p) (s c) -> p kp k2 s c", p=128, k2=2,
                          s=N_SG)
    w1_r = w1_d.rearrange("(k p) n -> p k n", p=128)    # [128, 6, 64]
    out_r = out_d.rearrange("(s c) -> s c", s=N_SG)     # [4, 2048]

    relu = mybir.ActivationFunctionType.Relu

    with tile.TileContext(nc) as tc:
        with (
            tc.tile_pool(name="consts", bufs=1) as cpool,
            tc.tile_pool(name="xin", bufs=8) as px,
            tc.tile_pool(name="osb", bufs=2) as posb,
            tc.tile_pool(name="psz", bufs=4, space="PSUM") as psz,
            tc.tile_pool(name="pss", bufs=2, space="PSUM") as pss,
        ):
            # consts issue on the scalar HWDGE queue so the big xt DMAs
            # (sync queue) start immediately
            w1t = cpool.tile([128, KCH, H], BF16, tag="w1t")
            nc.scalar.dma_start(w1t[:], w1_r[:])
            # tcat lives at base partition 64 so lhsT/rhs base partitions
            # match in the feature-accumulation matmul
            tcat_full = cpool.tile([HT, H], BF16, tag="tcat")
            tcat = tcat_full[H:HT, :]
            nc.scalar.dma_start(tcat, tcat_d[:])
            wsc = cpool.tile([HT, 1], BF16, tag="wsc")
            nc.scalar.dma_start(wsc[:], wsc_d[:])

            # ht holds the full per-core score-matmul rhs for all groups:
            # rows 0..63 = relu(z) written per group, rows 64..85 = the
            # host-computed one-hot/mask plane, DMA'd once.
            ht = cpool.tile([HT, MS], BF16, tag="ht")
            nc.scalar.dma_start(ht[H:HT, :], oh_d[:])

            osb_tiles = {}
            from collections import deque
            pending = deque()

            def emit_score(g):
                sc = pss.tile([1, GROUP], F32, tag="pss")
                goff = GROUP * g
                nc.tensor.matmul(sc[:], wsc[:], ht[:, goff:goff + GROUP],
                                 start=True, stop=True)
                sq = g // SG
                if g % SG == 0:
                    osb_t = posb.tile([1, SG * GROUP], F32, tag="osb")
                    osb_tiles[sq] = osb_t
                orow = osb_tiles[sq][0:1, GROUP * (g % SG):GROUP * (g % SG + 1)]
                if g % 2 == 0:
                    nc.vector.tensor_copy(orow, sc[:])
                else:
                    nc.scalar.copy(orow, sc[:])
                if g % SG == SG - 1:
                    nc.gpsimd.dma_start(out_r[sq:sq + 1, :],
                                        osb_tiles.pop(sq)[:])

            def load_sg(s):
                xts = []
                for kp in range(KCH // 2):
                    xk = px.tile([128, 2, SG * GROUP], BF16, tag="xin")
                    if s == 0:
                        # split so group 0's chunks land first
                        nc.sync.dma_start(xk[:, :, 0:GROUP],
                                          xt_r[:, kp, :, s, 0:GROUP])
                        nc.sync.dma_start(xk[:, :, GROUP:],
                                          xt_r[:, kp, :, s, GROUP:])
                    else:
                        nc.sync.dma_start(xk[:], xt_r[:, kp, :, s, :])
                    xts.append(xk)
                return xts

            sg_tiles = {0: load_sg(0), 1: load_sg(1)}
            for s in range(N_SG):
                if s + 2 < N_SG:
                    sg_tiles[s + 2] = load_sg(s + 2)
                xts = sg_tiles.pop(s)
                for gi in range(SG):
                    g = SG * s + gi
                    off = GROUP * gi
                    goff = GROUP * g
                    # score for g-2 issues here so the PE never waits on
                    # the ACT relu of the group it just accumulated
                    if len(pending) >= 2:
                        emit_score(pending.popleft())

                    zt = psz.tile([H, GROUP], F32, tag="psz")
                    for k in range(KCH):
                        nc.tensor.matmul(zt[:], w1t[:, k, :],
                                         xts[k // 2][:, k % 2,
                                                     off:off + GROUP],
                                         start=(k == 0), stop=False)
                    # feature/bias/mask contribution via the one-hot plane
                    nc.tensor.matmul(zt[:], tcat, ht[H:HT, goff:goff + GROUP],
                                     start=False, stop=True)

                    nc.scalar.activation(ht[0:H, goff:goff + GROUP], zt[:],
                                         relu)
                    pending.append(g)
            while pending:
                emit_score(pending.popleft())

    nc.compile()
    _CACHE["nc"] = nc
    return nc


def _bucket(c):
    """Reference get_bucket, replicated with the same XLA CPU float ops so
    boundary cases (c = 8, 16, 32) bucket identically."""
    import math
    import jax
    import jax.numpy as jnp
    cpu = jax.devices("cpu")[0]
    with jax.default_device(cpu):
        c = jnp.asarray(c).astype(jnp.int32)
        logspace = jnp.floor(
            jnp.log(jnp.maximum(c, 1).astype(jnp.float32)) / math.log(2)
        ).astype(jnp.int32) + 3
        idx = jnp.where(c <= 4, c, logspace)
        return np.asarray(jnp.clip(idx, 0, 9))


def _prepare_maps(ment_emb, mem_vectors, dist_table, counter_table,
                  W1, b1, W2, b2, ent_counter, last_mention_start, ment_start):
    import ml_dtypes
    f32 = np.float32
    bf16 = ml_dtypes.bfloat16
    ment = np.asarray(ment_emb, f32)
    mem = np.asarray(mem_vectors, f32)
    W1 = np.asarray(W1, f32)
    ms = int(np.asarray(ment_start))

    W1m, W1r, W1h = W1[0:D], W1[D:2 * D], W1[2 * D:3 * D]
    W1d, W1c = W1[3 * D:3 * D + E], W1[3 * D + E:3 * D + 2 * E]

    w1eff = (W1m + ment[:, None] * W1h).astype(f32)              # [768, 64]
    bias_vec = (np.asarray(b1, f32) + ment @ W1r).astype(f32)    # [64]
    T_d = (np.asarray(dist_table, f32) @ W1d + bias_vec).astype(f32)
    T_c = (np.asarray(counter_table, f32) @ W1c).astype(f32)
    b2v = float(np.asarray(b2, f32).reshape(-1)[0])

    tcat = np.concatenate(
        [T_d, T_c, np.full((1, H), -BIG, f32), np.zeros((1, H), f32)], 0)
    # single score matmul: rows 0..63 act on relu(z), rows 64..85 on onehot
    wsc = np.zeros((HT, 1), f32)
    wsc[0:H, 0] = np.asarray(W2, f32).reshape(-1)
    wsc[H + 20, 0] = -10000.0 - b2v
    wsc[H + 21, 0] = b2v

    cnt_i = np.asarray(ent_counter).astype(np.int64)
    dist_i = ms - np.asarray(last_mention_start).astype(np.int64)
    bd = _bucket(dist_i)                                         # [M] in 0..9
    bc = _bucket(cnt_i)                                          # [M] in 0..9
    r = np.arange(10)
    oh = np.empty((NF, M), f32)
    oh[0:10] = (bd[None, :] == r[:, None])
    oh[10:20] = (bc[None, :] == r[:, None])
    oh[20] = (cnt_i <= 0)
    oh[21] = 1.0
    oh = oh.astype(bf16)

    w1_b = w1eff.astype(bf16)
    tcat_b = tcat.astype(bf16)
    wsc_b = wsc.astype(bf16)

    in_maps = []
    for c in range(N_CORES):
        sl = slice(c * MS, (c + 1) * MS)
        in_maps.append(dict(
            xt=np.ascontiguousarray(mem[sl].T.astype(bf16)),
            oh=np.ascontiguousarray(oh[:, sl]),
            w1=w1_b, tcat=tcat_b, wsc=wsc_b))
    return in_maps


def _postprocess(results):
    out = np.empty(M + 1, np.float32)
    for c in range(N_CORES):
        out[c * MS:(c + 1) * MS] = results[c]["out"]
    out[M] = 0.0
    return out


def run_spmd(in_maps, trace=False):
    from concourse.bass_utils import run_bass_kernel_spmd
    nc = _build()
    return run_bass_kernel_spmd(nc, in_maps, list(range(N_CORES)), trace=trace)


def kernel(**inputs):
    in_maps = _prepare_maps(**inputs)
    res = run_spmd(in_maps, trace=False)
    return _postprocess(res.results)
